# revision 1
# baseline (speedup 1.0000x reference)
"""Trainium2 Bass kernel for gnn_message_passing (nn_CMMLunit_50173807952434).

reference math (per batch sample, N=4096, D=128, H=512, O=128):
    d2[i,j] = ||r_i||^2 + ||r_j||^2 - 2 r_i.r_j   (clamped at 0)
    w = exp(-d2); w = w / rowsum(w); w = w + I
    r2 = w @ r
    out = leaky_relu(r2 @ W1 + b1, 0.01) @ W2 + b2

Sharding: data-parallel over batch B=8 across 8 cores (1 sample/core),
FFN weights replicated, no collectives.

Per-core pipeline (all matmuls bf16 into fp32 PSUM):
  - load r -> r_bf [128,(nb,128)] ; rT_bf [128,N] via 32 DMA transposes
  - sq via DVE tensor_tensor_reduce (scale -0.5 => nhsq = -sq/2)
  - gram row-block n, column-tile q of 1024:
      PSUM g = rT_n.T @ rT_cols   (2 chunks of 512)
      sq_i/sq_j added either by a K=2 augmented matmul (PE) or by a fused
      DVE scalar_tensor_tensor (g + nhsq_i) + nhsq_bcast_j  -> -d2/2
      ACT: u = Exp(2 * (-d2/2)) bf16, accum_out -> row-sum slots
      yT[128,2048-half] += r_n.T?? no: yT accum: matmul(lhsT=r_n, rhs=u)
  - s = sum of slots; sinv broadcast to [128,N] via DRAM bounce;
    r2T = yT * sinv + rT  (bf16)
  - FFN: hT = max(v, 0.01v), v = W1.T@r2T + b1 (b1 via rank-1 matmul);
    out = hT.T@W2 + b2 (b2 via rank-1 matmul), DMA PSUM->DRAM.
"""

import numpy as np
from contextlib import ExitStack

import concourse.bass as bass
import concourse.bacc as bacc
import concourse.tile as tile
from concourse import mybir
from concourse.bass_utils import run_bass_kernel_spmd
from concourse.masks import make_identity

F32 = mybir.dt.float32
BF16 = mybir.dt.bfloat16
Alu = mybir.AluOpType
Act = mybir.ActivationFunctionType

P = 128  # partitions

# main problem dims (hardcoded; harness contract)
B_FULL, N_FULL, D_FULL = 8, 4096, 128
H_FULL, O_FULL = 512, 128
N_CORES = 8


def build_nc(
    N=N_FULL,
    D=D_FULL,
    H=H_FULL,
    O=O_FULL,
    aug_mod=1,
    use_dma_transpose=False,
    debug_stage=99,
):
    """Build the single-core Bass program (SPMD across cores)."""
    assert D == P
    NB = N // P              # row blocks
    HB = H // P
    QW = min(1024, N)        # gram/ACT tile width (<=2 psum banks)
    NPASS = N // QW          # column passes (yT psum [P, QW] per pass)
    CH = min(512, QW)        # matmul chunk (one psum bank)
    CPQ = QW // CH
    NSLOT = NPASS            # accum slots per row block

    nc = bacc.Bacc("TRN2", target_bir_lowering=False, debug=False)
    r_ext = nc.declare_dram_parameter("r", [N, D], F32, isOutput=False)
    w1_ext = nc.declare_dram_parameter("W1", [D, H], F32, isOutput=False)
    b1_ext = nc.declare_dram_parameter("b1", [H], F32, isOutput=False)
    w2_ext = nc.declare_dram_parameter("W2", [H, O], F32, isOutput=False)
    b2_ext = nc.declare_dram_parameter("b2", [O], F32, isOutput=False)
    out_ext = nc.declare_dram_parameter("out", [N, O], F32, isOutput=True)

    # DRAM bounce buffers (partition->free transposition staging)
    scr_nhsq = nc.dram_tensor("scr_nhsq", [NB, P], F32)
    scr_nhsq_bf = nc.dram_tensor("scr_nhsq_bf", [NB, P], BF16)
    scr_sq_bf = nc.dram_tensor("scr_sq_bf", [NB, P], BF16)
    scr_sinv = nc.dram_tensor("scr_sinv", [NB, P], F32)

    def flat_bcast_ap(dram_t, parts, n):
        # read [nb,p] dram tensor as a [parts, n] partition-broadcast AP
        a = dram_t[:, :].rearrange("a b -> (a b)")
        return bass.AP(tensor=a.tensor, offset=a.offset, ap=[[0, parts]] + list(a.ap))

    def flat_row_ap(dram_t):
        a = dram_t[:, :].rearrange("a b -> (a b)")
        return bass.AP(tensor=a.tensor, offset=a.offset, ap=[[1, 1]] + list(a.ap))

    with tile.TileContext(nc) as tc, ExitStack() as ctx:
        consts = ctx.enter_context(tc.tile_pool(name="consts", bufs=1))
        stage = ctx.enter_context(tc.tile_pool(name="stage", bufs=2))
        upool = ctx.enter_context(tc.tile_pool(name="upool", bufs=3))
        psA = ctx.enter_context(tc.tile_pool(name="psA", bufs=3, space="PSUM"))
        psY = ctx.enter_context(tc.tile_pool(name="psY", bufs=1, space="PSUM"))

        ident = consts.tile([P, P], F32)
        make_identity(nc, ident)

        # ---- load & cast inputs ------------------------------------------
        r_bf = consts.tile([P, NB, D], BF16)
        rT_bf = consts.tile([P, N], BF16)
        for b in range(NB):
            rld = upool.tile([P, D], F32, tag="rld")
            dma_eng = nc.sync if b % 2 == 0 else nc.scalar
            dma_eng.dma_start(out=rld, in_=r_ext[b * P : (b + 1) * P, :])
            nc.vector.tensor_copy(out=r_bf[:, b, :], in_=rld)
            if use_dma_transpose:
                nc.sync.dma_start_transpose(
                    out=rT_bf[:, b * P : (b + 1) * P], in_=r_bf[:, b, :]
                )
            else:
                tp = psA.tile([P, QW], F32, tag="ps")
                nc.tensor.transpose(tp[:, :P], rld, ident)
                nc.scalar.copy(out=rT_bf[:, b * P : (b + 1) * P], in_=tp[:, :P])

        w1f = consts.tile([P, H], F32)
        nc.gpsimd.dma_start(out=w1f, in_=w1_ext[:, :])
        w1_bf = consts.tile([P, H], BF16)
        nc.vector.tensor_copy(out=w1_bf, in_=w1f)

        b1f = consts.tile([1, H], F32)
        nc.gpsimd.dma_start(out=b1f, in_=b1_ext[:][None, :])
        b1_bf = consts.tile([1, H], BF16)
        nc.vector.tensor_copy(out=b1_bf, in_=b1f)

        w2f = consts.tile([P, HB, O], F32)
        nc.gpsimd.dma_start(out=w2f, in_=w2_ext[:, :].rearrange("(hb p) o -> p hb o", p=P))
        w2_bf = consts.tile([P, HB, O], BF16)
        nc.vector.tensor_copy(out=w2_bf, in_=w2f)

        b2f = consts.tile([1, O], F32)
        nc.gpsimd.dma_start(out=b2f, in_=b2_ext[:][None, :])
        b2_bf = consts.tile([1, O], BF16)
        nc.vector.tensor_copy(out=b2_bf, in_=b2f)

        ones_bf = consts.tile([1, CH], BF16)
        nc.gpsimd.memset(ones_bf, 1.0)

        # ---- sq machinery ------------------------------------------------
        # nhsq_col[:, b] = -0.5 * sum_d r_bf[p, b, d]^2   (matches bf16 gram)
        # (tensor_tensor_reduce is a custom-library DVE op that fails at
        #  runtime under this PJRT path; use standard tt + reduce instead)
        sq_col = consts.tile([P, NB], F32)
        for b in range(NB):
            rsq = upool.tile([P, D], BF16, tag="rsq")
            # Square(r * sqrt(0.5)) = 0.5*r^2; accum -> sq/2 per partition
            nc.scalar.activation(
                out=rsq,
                in_=r_bf[:, b, :],
                func=Act.Square,
                bias=0.0,
                scale=0.70710678,
                accum_out=sq_col[:, b : b + 1],
            )
        nhsq_col = consts.tile([P, NB], F32)
        nc.vector.tensor_scalar_mul(nhsq_col, sq_col, -1.0)

        # transpose nhsq_col -> [NB, P] and bounce through DRAM to build
        # row-layout copies: aug rows and the [P, N] broadcast tile.
        tpq = psA.tile([P, QW], F32, tag="ps")
        nc.tensor.transpose(tpq[:NB, :P], nhsq_col, ident)
        nhsqT_f = stage.tile([NB, P], F32)
        nc.vector.tensor_copy(out=nhsqT_f, in_=tpq[:NB, :P])
        nhsqT_bf = stage.tile([NB, P], BF16)
        nc.vector.tensor_copy(out=nhsqT_bf, in_=tpq[:NB, :P])
        sqT_bf = stage.tile([NB, P], BF16)
        nc.vector.tensor_scalar_mul(sqT_bf, tpq[:NB, :P], -2.0)
        nc.sync.dma_start(out=scr_nhsq[:, :], in_=nhsqT_f)
        nc.sync.dma_start(out=scr_nhsq_bf[:, :], in_=nhsqT_bf)
        nc.sync.dma_start(out=scr_sq_bf[:, :], in_=sqT_bf)

        # augmented-matmul operands, paired by k-row:
        #   k=0: augL -0.5 const   x augR sq_j
        #   k=1: augL -sq_i/2      x augR 1.0 const
        # engine ops can't start at partition 1, so partition-1 rows are
        # filled by DMA (from partition-0 staging tiles).
        augL = consts.tile([2, N], BF16)
        augR = consts.tile([2, N], BF16)
        nc.gpsimd.memset(augL[0:1, :], -0.5)
        onesN = consts.tile([1, N], BF16)
        nc.gpsimd.memset(onesN, 1.0)
        nc.sync.dma_start(out=augL[1:2, :], in_=flat_row_ap(scr_nhsq_bf))
        nc.sync.dma_start(out=augR[0:1, :], in_=flat_row_ap(scr_sq_bf))
        nc.sync.dma_start(out=augR[1:2, :], in_=onesN)

        nhsq_bcast = consts.tile([P, N], F32)
        if aug_mod != 1:
            bcn = flat_bcast_ap(scr_nhsq, P, N)
            engs = [nc.gpsimd, nc.sync, nc.scalar]
            for qp in range(NPASS):
                chunk_ap = bass.AP(
                    tensor=bcn.tensor,
                    offset=bcn.offset + qp * QW,
                    ap=[[0, P], [1, QW]],
                )
                engs[qp % 3].dma_start(
                    out=nhsq_bcast[:, qp * QW : (qp + 1) * QW], in_=chunk_ap
                )

        def dbg_out():
            for b in range(NB):
                dt = upool.tile([P, D], F32, tag="dbg")
                nc.vector.tensor_copy(out=dt, in_=r_bf[:, b, :])
                nc.sync.dma_start(out=out_ext[b * P : (b + 1) * P, :], in_=dt)

        if debug_stage < 2:
            dbg_out()

        if debug_stage >= 2:
            # ---- main loop: gram -> exp -> aggregate -------------------------
            s_slots = consts.tile([P, NB * NSLOT], F32)
            ysb = consts.tile([P, N], F32)

            for qp in range(NPASS):
                base = qp * QW
                yt = psY.tile([P, QW], F32, tag="y")
                for n in range(NB):
                    aug = aug_mod > 0 and (n % aug_mod == 0)
                    ncol = slice(n * P, (n + 1) * P)
                    g = psA.tile([P, QW], F32, tag="ps")
                    for c in range(CPQ):
                        cs = slice(c * CH, (c + 1) * CH)
                        rcol = slice(base + c * CH, base + (c + 1) * CH)
                        nc.tensor.matmul(
                            g[:, cs],
                            lhsT=rT_bf[:, ncol],
                            rhs=rT_bf[:, rcol],
                            start=True,
                            stop=not aug,
                        )
                        if aug:
                            nc.tensor.matmul(
                                g[:, cs],
                                lhsT=augL[:, ncol],
                                rhs=augR[:, rcol],
                                start=False,
                                stop=True,
                            )
                    slot = n * NSLOT + qp
                    u = upool.tile([P, QW], BF16, tag="u")
                    if aug:
                        nc.scalar.activation(
                            out=u,
                            in_=g,
                            func=Act.Exp,
                            bias=0.0,
                            scale=2.0,
                            accum_out=s_slots[:, slot : slot + 1],
                        )
                    else:
                        d2 = upool.tile([P, QW], BF16, tag="d2")
                        nc.vector.scalar_tensor_tensor(
                            out=d2,
                            in0=g,
                            scalar=nhsq_col[:, n : n + 1],
                            in1=nhsq_bcast[:, base : base + QW],
                            op0=Alu.add,
                            op1=Alu.add,
                        )
                        nc.scalar.activation(
                            out=u,
                            in_=d2,
                            func=Act.Exp,
                            bias=0.0,
                            scale=2.0,
                            accum_out=s_slots[:, slot : slot + 1],
                        )
                    for c in range(CPQ):
                        cs = slice(c * CH, (c + 1) * CH)
                        nc.tensor.matmul(
                            yt[:, cs],
                            lhsT=r_bf[:, n, :],
                            rhs=u[:, cs],
                            start=(n == 0),
                            stop=(n == NB - 1),
                        )
                nc.vector.tensor_copy(out=ysb[:, base : base + QW], in_=yt)

        if debug_stage < 3 and debug_stage >= 2:
            dbg_out()

        if debug_stage >= 3:
            # warm-keeper: the PE would otherwise idle >3.4us here (waiting on
            # the row-sum -> 1/s broadcast chain) and the clock gate would
            # re-throttle it to 1.2 GHz for the whole FFN. Keep it busy with a
            # throwaway accumulation; one tiny consumer DMA keeps it live.
            NDUMMY = 40
            dummy_ps = psY.tile([P, CH], F32, tag="y")
            for i in range(NDUMMY):
                nc.tensor.matmul(
                    dummy_ps,
                    lhsT=rT_bf[:, 0:P],
                    rhs=rT_bf[:, 0:CH],
                    start=(i == 0),
                    stop=(i == NDUMMY - 1),
                )
            dsb = stage.tile([1, 8], F32)
            nc.vector.tensor_copy(out=dsb, in_=dummy_ps[0:1, 0:8])
            nc.sync.dma_start(out=scr_nhsq[0:1, 0:8], in_=dsb)

            # ---- normalize + residual ----------------------------------------
            s_col = consts.tile([P, NB], F32)
            if NSLOT == 1:
                nc.vector.tensor_copy(out=s_col, in_=s_slots)
            elif NSLOT == 2:
                nc.vector.tensor_tensor(
                    out=s_col,
                    in0=s_slots.rearrange("p (nb t) -> p nb t", t=2)[:, :, 0],
                    in1=s_slots.rearrange("p (nb t) -> p nb t", t=2)[:, :, 1],
                    op=Alu.add,
                )
            else:
                nc.vector.tensor_reduce(
                    out=s_col,
                    in_=s_slots.rearrange("p (nb t) -> p nb t", t=NSLOT),
                    axis=mybir.AxisListType.X,
                    op=Alu.add,
                )
            sinv_col = consts.tile([P, NB], F32)
            nc.vector.reciprocal(out=sinv_col, in_=s_col)
            tps = psA.tile([P, QW], F32, tag="ps")
            nc.tensor.transpose(tps[:NB, :P], sinv_col, ident)
            sinvT_f = stage.tile([NB, P], F32)
            nc.vector.tensor_copy(out=sinvT_f, in_=tps[:NB, :P])
            nc.sync.dma_start(out=scr_sinv[:, :], in_=sinvT_f)
            # chunked broadcast + normalize so the FFN can start on chunk 0
            # while later chunks are in flight (shrinks the PE idle bubble
            # that would otherwise re-throttle the PE clock mid-kernel).
            sinv_bcast = consts.tile([P, N], F32)
            r2 = consts.tile([P, N], BF16)
            bc = flat_bcast_ap(scr_sinv, P, N)
            for qp in range(NPASS):
                cs = slice(qp * QW, (qp + 1) * QW)
                chunk_ap = bass.AP(
                    tensor=bc.tensor,
                    offset=bc.offset + qp * QW,
                    ap=[[0, P], [1, QW]],
                )
                (nc.sync if qp % 2 == 0 else nc.scalar).dma_start(
                    out=sinv_bcast[:, cs], in_=chunk_ap
                )
                nc.vector.tensor_tensor(
                    out=r2[:, cs], in0=ysb[:, cs], in1=sinv_bcast[:, cs],
                    op=Alu.mult,
                )
                nc.vector.tensor_tensor(
                    out=r2[:, cs], in0=r2[:, cs], in1=rT_bf[:, cs], op=Alu.add
                )

        if debug_stage < 4 and debug_stage >= 3:
            dbg_out()

        if debug_stage >= 4:
            # ---- FFN ----------------------------------------------------------
            hT = [consts.tile([P, N], BF16, name=f"hT{hb}", tag=f"hT{hb}") for hb in range(HB)]
            for hb in range(HB):
                hcol = slice(hb * P, (hb + 1) * P)
                for seg in range(N // QW):
                    hp = psA.tile([P, QW], F32, tag="ps")
                    for c in range(CPQ):
                        cs = slice(c * CH, (c + 1) * CH)
                        rcol = slice(seg * QW + c * CH, seg * QW + (c + 1) * CH)
                        nc.tensor.matmul(
                            hp[:, cs],
                            lhsT=b1_bf[0:1, hcol],
                            rhs=ones_bf[0:1, :CH],
                            start=True,
                            stop=False,
                        )
                        nc.tensor.matmul(
                            hp[:, cs],
                            lhsT=w1_bf[:, hcol],
                            rhs=r2[:, rcol],
                            start=False,
                            stop=True,
                        )
                    # leaky relu: max(v, 0.01*v). stt cannot read PSUM twice,
                    # so stage v through SBUF via an ACT copy first.
                    v = upool.tile([P, QW], BF16, tag="v")
                    nc.scalar.copy(out=v, in_=hp)
                    nc.vector.scalar_tensor_tensor(
                        out=hT[hb][:, seg * QW : (seg + 1) * QW],
                        in0=v,
                        scalar=0.01,
                        in1=v,
                        op0=Alu.mult,
                        op1=Alu.max,
                    )

            for nb in range(NB):
                op = psA.tile([P, O], F32, tag="ps")
                nc.tensor.matmul(
                    op,
                    lhsT=ones_bf[0:1, :P],
                    rhs=b2_bf[0:1, :],
                    start=True,
                    stop=False,
                )
                for hb in range(HB):
                    nc.tensor.matmul(
                        op,
                        lhsT=hT[hb][:, nb * P : (nb + 1) * P],
                        rhs=w2_bf[:, hb, :],
                        start=False,
                        stop=(hb == HB - 1),
                    )
                osb = upool.tile([P, O], F32, tag="osb")
                nc.scalar.copy(out=osb, in_=op)
                (nc.sync if nb % 2 == 0 else nc.scalar).dma_start(
                    out=out_ext[nb * P : (nb + 1) * P, :], in_=osb
                )

    nc.compile()
    return nc


_NC_CACHE = {}


def _get_nc(**kw):
    key = tuple(sorted(kw.items()))
    if key not in _NC_CACHE:
        _NC_CACHE[key] = build_nc(**kw)
    return _NC_CACHE[key]


def kernel(r, W1, b1, W2, b2):
    r = np.ascontiguousarray(r, dtype=np.float32)
    W1 = np.ascontiguousarray(W1, dtype=np.float32)
    b1 = np.ascontiguousarray(b1, dtype=np.float32)
    W2 = np.ascontiguousarray(W2, dtype=np.float32)
    b2 = np.ascontiguousarray(b2, dtype=np.float32)
    B, N, D = r.shape
    assert (B, N, D) == (B_FULL, N_FULL, D_FULL)

    nc = _get_nc()
    in_maps = [
        {"r": r[i], "W1": W1, "b1": b1, "W2": W2, "b2": b2} for i in range(B)
    ]
    res = run_bass_kernel_spmd(nc, in_maps, list(range(N_CORES)))
    return np.stack([res.results[i]["out"] for i in range(B)]).astype(np.float32)


if __name__ == "__main__":
    rng = np.random.default_rng(0)
    r = rng.standard_normal((B_FULL, N_FULL, D_FULL), dtype=np.float32)
    W1 = rng.standard_normal((D_FULL, H_FULL), dtype=np.float32) * 0.08
    b1 = rng.standard_normal((H_FULL,), dtype=np.float32) * 0.08
    W2 = rng.standard_normal((H_FULL, O_FULL), dtype=np.float32) * 0.04
    b2 = rng.standard_normal((O_FULL,), dtype=np.float32) * 0.04
    out = kernel(r=r, W1=W1, b1=b1, W2=W2, b2=b2)
    print(out.shape, out.dtype)



# revision 12
# speedup vs baseline: 1.1323x; 1.1323x over previous
"""Trainium2 Bass kernel for gnn_message_passing (nn_CMMLunit_50173807952434).

reference math (per batch sample, N=4096, D=128, H=512, O=128):
    d2[i,j] = ||r_i||^2 + ||r_j||^2 - 2 r_i.r_j   (clamped at 0)
    w = exp(-d2); w = w / rowsum(w); w = w + I
    r2 = w @ r
    out = leaky_relu(r2 @ W1 + b1, 0.01) @ W2 + b2

Sharding: data-parallel over batch B=8 across 8 cores (1 sample/core),
FFN weights replicated, no collectives.

v2 per-core pipeline (all matmuls bf16 into fp32 PSUM):
  - load r -> r_bf [128,(nb,128)]; rT_bf [128,N] via PE transposes
  - sq/2 via ACT Square accum; -sq_j/2 row [1,N] via 16KB DRAM bounce
  - per column pass qp (4 x 1024 cols):
      bcq[128,1024] = broadcast(-sq_j/2) via rank-1 PE matmul (once/pass)
      per row block n (32): PSUM g = rT_n.T @ rT_cols (2x512, one shared
        stationary); DVE tt: d2h = g + bcq (bf16); ACT: u =
        Exp(2*d2h - sq_i) with per-partition bias, accum -> row-sum slot;
        yT += r_n.T @ u (software-pipelined 2 iters behind gram)
  - s = sum slots; 1/s -> row [1,N] via 16KB bounce; rank-1 PE broadcast
    per chunk; r2T = yT*sinv + rT (DVE)
  - FFN: hT = Lrelu(W1.T@r2T + b1) via ACT bias+alpha;
    outT[o,n] = sum_hb W2_hb.T @ hT_hb (W2 stationary), +b2 via DVE,
    DMA outT [O,N] -> DRAM; host transposes.
"""

import numpy as np
from contextlib import ExitStack

import concourse.bass as bass
import concourse.bacc as bacc
import concourse.tile as tile
from concourse import mybir
from concourse.bass_utils import run_bass_kernel_spmd
from concourse.masks import make_identity

F32 = mybir.dt.float32
BF16 = mybir.dt.bfloat16
Alu = mybir.AluOpType
Act = mybir.ActivationFunctionType

P = 128  # partitions

# main problem dims (hardcoded; harness contract)
B_FULL, N_FULL, D_FULL = 8, 4096, 128
H_FULL, O_FULL = 512, 128
N_CORES = 8

USE_ACT_LRELU = True  # Lrelu on ACT (bias=b1, alpha=0.01); else copy+DVE stt


def build_nc(N=N_FULL, D=D_FULL, H=H_FULL, O=O_FULL):
    """Build the single-core Bass program (SPMD across cores)."""
    assert D == P
    NB = N // P              # row blocks
    HB = H // P
    QW = min(1024, N)        # gram/ACT tile width (2 psum banks)
    NPASS = N // QW          # column passes
    CH = 512                 # matmul chunk (one psum bank)
    CPQ = QW // CH
    LA = 2                   # y-matmul lookahead (software pipeline depth)

    nc = bacc.Bacc("TRN2", target_bir_lowering=False, debug=False)
    r_ext = nc.declare_dram_parameter("r", [N, D], F32, isOutput=False)
    w1_ext = nc.declare_dram_parameter("W1", [D, H], F32, isOutput=False)
    b1_ext = nc.declare_dram_parameter("b1", [H], F32, isOutput=False)
    w2_ext = nc.declare_dram_parameter("W2", [H, O], F32, isOutput=False)
    b2_ext = nc.declare_dram_parameter("b2", [O], F32, isOutput=False)
    # transposed bf16 output [O, N]; host transposes + upcasts
    out_ext = nc.declare_dram_parameter("out", [O, N], BF16, isOutput=True)

    # DRAM bounce buffers (partition->free transposition staging, 16KB each)
    scr_nhsq_bf = nc.dram_tensor("scr_nhsq_bf", [NB, P], BF16)
    scr_sinv = nc.dram_tensor("scr_sinv", [NB, P], F32)

    def flat_row_ap(dram_t):
        a = dram_t[:, :].rearrange("a b -> (a b)")
        return bass.AP(tensor=a.tensor, offset=a.offset, ap=[[1, 1]] + list(a.ap))

    def col_ap(dram_1d, parts, nfree):
        # read 1-D dram tensor [parts*nfree] as [parts, nfree] column layout:
        # out[p, f] = t[f*parts + p]
        a = dram_1d[:]
        return bass.AP(
            tensor=a.tensor, offset=a.offset, ap=[[1, parts], [parts, nfree]]
        )

    with tile.TileContext(nc) as tc, ExitStack() as ctx:
        consts = ctx.enter_context(tc.tile_pool(name="consts", bufs=1))
        stage = ctx.enter_context(tc.tile_pool(name="stage", bufs=2))
        upool = ctx.enter_context(tc.tile_pool(name="upool", bufs=3))
        psA = ctx.enter_context(tc.tile_pool(name="psA", bufs=2, space="PSUM"))
        psY = ctx.enter_context(tc.tile_pool(name="psY", bufs=1, space="PSUM"))
        psO = ctx.enter_context(tc.tile_pool(name="psO", bufs=2, space="PSUM"))

        ident = consts.tile([P, P], F32)
        make_identity(nc, ident)

        ones_bf = consts.tile([1, P], BF16)
        nc.gpsimd.memset(ones_bf, 1.0)
        ones_f = consts.tile([1, P], F32)
        nc.gpsimd.memset(ones_f, 1.0)

        # ---- load & cast inputs ------------------------------------------
        # 4-block DMA groups across 4 queues; transposes on PE; psum copies
        # alternate DVE/ACT; Square (for sq/2) reads the bf16 cast so the
        # gram diagonal matches the bf16 matmul.
        GB = 4
        r_bf = consts.tile([P, NB, D], BF16)
        rT_bf = consts.tile([P, N], BF16)
        sqh_col = consts.tile([P, NB], F32)   # +sq/2
        r_src = r_ext[:, :].rearrange("(nb p) d -> p nb d", p=P)
        dqs = [nc.sync, nc.gpsimd, nc.scalar]
        for g0 in range(0, NB, GB):
            rld = upool.tile([P, GB, D], F32, tag="rld")
            dqs[(g0 // GB) % 3].dma_start(
                out=rld, in_=r_src[:, g0 : g0 + GB, :]
            )
            nc.vector.tensor_copy(out=r_bf[:, g0 : g0 + GB, :], in_=rld)
            for bi in range(GB):
                b = g0 + bi
                tp = psA.tile([P, QW], F32, tag="ps")
                nc.tensor.transpose(tp[:, :P], rld[:, bi, :], ident)
                if b % 2 == 0:
                    nc.vector.tensor_copy(
                        out=rT_bf[:, b * P : (b + 1) * P], in_=tp[:, :P]
                    )
                else:
                    nc.scalar.copy(
                        out=rT_bf[:, b * P : (b + 1) * P], in_=tp[:, :P]
                    )
                rsq = upool.tile([P, D], BF16, tag="rsq")
                # Square(r * sqrt(0.5)) = 0.5*r^2; accum -> sq/2 per partition
                nc.scalar.activation(
                    out=rsq,
                    in_=r_bf[:, b, :],
                    func=Act.Square,
                    bias=0.0,
                    scale=0.70710678,
                    accum_out=sqh_col[:, b : b + 1],
                )

        # FFN weights (replicated, small): loads on gpsimd queue
        w1f = stage.tile([P, H], F32, tag="wld")
        nc.gpsimd.dma_start(out=w1f, in_=w1_ext[:, :])
        w1_bf = consts.tile([P, H], BF16)
        nc.vector.tensor_copy(out=w1_bf, in_=w1f)

        w2f = stage.tile([P, HB, O], F32, tag="wld2")
        nc.gpsimd.dma_start(
            out=w2f, in_=w2_ext[:, :].rearrange("(hb p) o -> p hb o", p=P)
        )
        w2_bf = consts.tile([P, HB, O], BF16)
        nc.vector.tensor_copy(out=w2_bf, in_=w2f)

        b1_col = consts.tile([P, HB], F32)    # b1[hb*128+p]
        nc.gpsimd.dma_start(out=b1_col, in_=col_ap(b1_ext, P, HB))
        b2_col = consts.tile([P, 1], F32)
        nc.gpsimd.dma_start(out=b2_col, in_=col_ap(b2_ext, P, 1))

        # ---- sq machinery ------------------------------------------------
        nsq_col = consts.tile([P, NB], F32)   # -sq (Exp bias)
        nc.vector.tensor_scalar_mul(nsq_col, sqh_col, -2.0)
        nhsq_col = consts.tile([P, NB], F32)  # -sq/2
        nc.vector.tensor_scalar_mul(nhsq_col, sqh_col, -1.0)

        # transpose -sq/2 -> [NB, P], bounce 16KB through DRAM -> row [1, N]
        tpq = psA.tile([P, QW], F32, tag="ps")
        nc.tensor.transpose(tpq[:NB, :P], nhsq_col, ident)
        nhsqT_bf = stage.tile([NB, P], BF16, tag="nhsqT")
        nc.vector.tensor_copy(out=nhsqT_bf, in_=tpq[:NB, :P])
        nc.sync.dma_start(out=scr_nhsq_bf[:, :], in_=nhsqT_bf)
        nrow = consts.tile([1, N], BF16)
        nc.sync.dma_start(out=nrow, in_=flat_row_ap(scr_nhsq_bf))

        # ---- main loop: gram -> exp -> aggregate -------------------------
        s_slots = consts.tile([P, NB * NPASS], F32)
        ysb = consts.tile([P, N], F32)

        for qp in range(NPASS):
            base = qp * QW
            # bcq[p, j] = -sq_j/2 broadcast over partitions (rank-1 matmul)
            bc_ps = psA.tile([P, QW], F32, tag="ps")
            for c in range(CPQ):
                cs = slice(c * CH, (c + 1) * CH)
                nc.tensor.matmul(
                    bc_ps[:, cs],
                    lhsT=ones_bf,
                    rhs=nrow[0:1, base + c * CH : base + (c + 1) * CH],
                    start=True,
                    stop=True,
                )
            bcq = stage.tile([P, QW], F32, tag="bcq")
            nc.scalar.copy(out=bcq, in_=bc_ps)

            yt = psY.tile([P, QW], F32, tag="y")
            gtiles = [None] * NB
            utiles = [None] * NB

            def issue_gram(n):
                g = psA.tile([P, QW], F32, tag="ps")
                gtiles[n] = g
                ncol = slice(n * P, (n + 1) * P)
                for c in range(CPQ):
                    cs = slice(c * CH, (c + 1) * CH)
                    nc.tensor.matmul(
                        g[:, cs],
                        lhsT=rT_bf[:, ncol],
                        rhs=rT_bf[:, base + c * CH : base + (c + 1) * CH],
                        start=True,
                        stop=True,
                    )
                # in-place: g += -sq_j/2 (keeps Exp reading PSUM f32)
                nc.vector.tensor_tensor(out=g, in0=g, in1=bcq, op=Alu.add)
                u = upool.tile([P, QW], BF16, tag="u")
                utiles[n] = u
                slot = n * NPASS + qp
                nc.scalar.activation(
                    out=u,
                    in_=g,
                    func=Act.Exp,
                    bias=nsq_col[:, n : n + 1],
                    scale=2.0,
                    accum_out=s_slots[:, slot : slot + 1],
                )

            def issue_y(n):
                u = utiles[n]
                for c in range(CPQ):
                    cs = slice(c * CH, (c + 1) * CH)
                    nc.tensor.matmul(
                        yt[:, cs],
                        lhsT=r_bf[:, n, :],
                        rhs=u[:, cs],
                        start=(n == 0),
                        stop=(n == NB - 1),
                    )

            for k in range(NB + LA):
                if k < NB:
                    issue_gram(k)
                if k >= LA:
                    issue_y(k - LA)
            nc.scalar.copy(out=ysb[:, base : base + QW], in_=yt)

        # ---- normalize + residual ----------------------------------------
        s_col = consts.tile([P, NB], F32)
        nc.vector.tensor_reduce(
            out=s_col,
            in_=s_slots.rearrange("p (nb t) -> p nb t", t=NPASS),
            axis=mybir.AxisListType.X,
            op=Alu.add,
        )
        sinv_col = consts.tile([P, NB], F32)
        nc.vector.reciprocal(out=sinv_col, in_=s_col)
        tps = psA.tile([P, QW], F32, tag="ps")
        nc.tensor.transpose(tps[:NB, :P], sinv_col, ident)
        sinvT_f = stage.tile([NB, P], F32, tag="sinvT")
        nc.vector.tensor_copy(out=sinvT_f, in_=tps[:NB, :P])
        nc.sync.dma_start(out=scr_sinv[:, :], in_=sinvT_f)
        srow = consts.tile([1, N], F32)
        nc.sync.dma_start(out=srow, in_=flat_row_ap(scr_sinv))

        r2 = consts.tile([P, N], BF16)
        hT = [
            consts.tile([P, N], BF16, name=f"hT{hb}", tag=f"hT{hb}")
            for hb in range(HB)
        ]

        for qp in range(NPASS):
            base = qp * QW
            # sinv broadcast chunk via rank-1 matmul (f32)
            sb_ps = psA.tile([P, QW], F32, tag="ps")
            for c in range(CPQ):
                cs = slice(c * CH, (c + 1) * CH)
                nc.tensor.matmul(
                    sb_ps[:, cs],
                    lhsT=ones_f,
                    rhs=srow[0:1, base + c * CH : base + (c + 1) * CH],
                    start=True,
                    stop=True,
                )
            # r2 per 512-chunk so fc1 can start on chunk 0 early
            r2t = stage.tile([P, QW], BF16, tag="r2t")
            for c in range(CPQ):
                cs = slice(base + c * CH, base + (c + 1) * CH)
                lcs = slice(c * CH, (c + 1) * CH)
                nc.vector.tensor_tensor(
                    out=r2t[:, lcs], in0=ysb[:, cs], in1=sb_ps[:, lcs],
                    op=Alu.mult,
                )
                nc.vector.tensor_tensor(
                    out=r2[:, cs], in0=r2t[:, lcs], in1=rT_bf[:, cs],
                    op=Alu.add,
                )

            # fc1: per-chunk MMs gated only on their r2 chunk
            for hb in range(HB):
                hp = psA.tile([P, QW], F32, tag="ps")
                for c in range(CPQ):
                    cs = slice(base + c * CH, base + (c + 1) * CH)
                    nc.tensor.matmul(
                        hp[:, c * CH : (c + 1) * CH],
                        lhsT=w1_bf[:, hb * P : (hb + 1) * P],
                        rhs=r2[:, cs],
                        start=True,
                        stop=True,
                    )
                nc.scalar.activation(
                    out=hT[hb][:, base : base + QW],
                    in_=hp,
                    func=Act.Lrelu,
                    bias=b1_col[:, hb : hb + 1],
                    scale=1.0,
                    alpha=0.01,
                )

            # fc2 for this chunk: outT[o, n] = sum_hb W2_hb.T @ hT_hb
            for c in range(CPQ):
                ncols = slice(base + c * CH, base + (c + 1) * CH)
                op = psO.tile([P, CH], F32, tag="o")
                for hb in range(HB):
                    nc.tensor.matmul(
                        op,
                        lhsT=w2_bf[:, hb, :],
                        rhs=hT[hb][:, ncols],
                        start=(hb == 0),
                        stop=(hb == HB - 1),
                    )
                osb = upool.tile([P, CH], BF16, tag="osb")
                nc.vector.tensor_scalar_add(osb, op, b2_col[:, 0:1])
                (nc.sync if c % 2 == 0 else nc.gpsimd).dma_start(
                    out=out_ext[:, ncols], in_=osb
                )

    nc.compile()
    return nc


_NC_CACHE = {}


def _get_nc(**kw):
    key = tuple(sorted(kw.items()))
    if key not in _NC_CACHE:
        _NC_CACHE[key] = build_nc(**kw)
    return _NC_CACHE[key]


def kernel(r, W1, b1, W2, b2):
    r = np.ascontiguousarray(r, dtype=np.float32)
    W1 = np.ascontiguousarray(W1, dtype=np.float32)
    b1 = np.ascontiguousarray(b1, dtype=np.float32)
    W2 = np.ascontiguousarray(W2, dtype=np.float32)
    b2 = np.ascontiguousarray(b2, dtype=np.float32)
    B, N, D = r.shape
    assert (B, N, D) == (B_FULL, N_FULL, D_FULL)

    nc = _get_nc()
    in_maps = [
        {"r": r[i], "W1": W1, "b1": b1, "W2": W2, "b2": b2} for i in range(B)
    ]
    res = run_bass_kernel_spmd(nc, in_maps, list(range(N_CORES)))
    # out is bf16 [O, N] per core; transpose back + upcast
    return np.stack(
        [np.ascontiguousarray(res.results[i]["out"].astype(np.float32).T)
         for i in range(B)]
    )


if __name__ == "__main__":
    rng = np.random.default_rng(0)
    r = rng.standard_normal((B_FULL, N_FULL, D_FULL), dtype=np.float32)
    W1 = rng.standard_normal((D_FULL, H_FULL), dtype=np.float32) * 0.08
    b1 = rng.standard_normal((H_FULL,), dtype=np.float32) * 0.08
    W2 = rng.standard_normal((H_FULL, O_FULL), dtype=np.float32) * 0.04
    b2 = rng.standard_normal((O_FULL,), dtype=np.float32) * 0.04
    out = kernel(r=r, W1=W1, b1=b1, W2=W2, b2=b2)
    print(out.shape, out.dtype)


# revision 19
# speedup vs baseline: 1.6523x; 1.4593x over previous
"""Trainium2 Bass kernel for gnn_message_passing (nn_CMMLunit_50173807952434).

reference math (per batch sample, N=4096, D=128, H=512, O=128):
    d2[i,j] = ||r_i||^2 + ||r_j||^2 - 2 r_i.r_j   (clamped at 0)
    w = exp(-d2); w = w / rowsum(w); w = w + I
    r2 = w @ r
    out = leaky_relu(r2 @ W1 + b1, 0.01) @ W2 + b2

Sharding: data-parallel over batch B=8 across 8 cores (1 sample/core),
FFN weights replicated, no collectives.

v2 per-core pipeline (all matmuls bf16 into fp32 PSUM):
  - load r -> r_bf [128,(nb,128)]; rT_bf [128,N] via PE transposes
  - sq/2 via ACT Square accum; -sq_j/2 row [1,N] via 16KB DRAM bounce
  - per column pass qp (4 x 1024 cols):
      bcq[128,1024] = broadcast(-sq_j/2) via rank-1 PE matmul (once/pass)
      per row block n (32): PSUM g = rT_n.T @ rT_cols (2x512, one shared
        stationary); DVE tt: d2h = g + bcq (bf16); ACT: u =
        Exp(2*d2h - sq_i) with per-partition bias, accum -> row-sum slot;
        yT += r_n.T @ u (software-pipelined 2 iters behind gram)
  - s = sum slots; 1/s -> row [1,N] via 16KB bounce; rank-1 PE broadcast
    per chunk; r2T = yT*sinv + rT (DVE)
  - FFN: hT = Lrelu(W1.T@r2T + b1) via ACT bias+alpha;
    outT[o,n] = sum_hb W2_hb.T @ hT_hb (W2 stationary), +b2 via DVE,
    DMA outT [O,N] -> DRAM; host transposes.
"""

import numpy as np
from contextlib import ExitStack

import concourse.bass as bass
import concourse.bacc as bacc
import concourse.tile as tile
from concourse import mybir
from concourse.bass_utils import run_bass_kernel_spmd
from concourse.masks import make_identity

F32 = mybir.dt.float32
BF16 = mybir.dt.bfloat16
Alu = mybir.AluOpType
Act = mybir.ActivationFunctionType

P = 128  # partitions

# main problem dims (hardcoded; harness contract)
B_FULL, N_FULL, D_FULL = 8, 4096, 128
H_FULL, O_FULL = 512, 128
N_CORES = 8

USE_ACT_LRELU = True  # Lrelu on ACT (bias=b1, alpha=0.01); else copy+DVE stt


def build_nc(N=N_FULL, D=D_FULL, H=H_FULL, O=O_FULL):
    """Build the single-core Bass program (SPMD across cores)."""
    assert D == P
    NB = N // P              # row blocks
    HB = H // P
    QW = min(1024, N)        # gram/ACT tile width (2 psum banks)
    NPASS = N // QW          # column passes
    CH = 512                 # matmul chunk (one psum bank)
    CPQ = QW // CH
    LA = 2                   # y-matmul lookahead (software pipeline depth)

    nc = bacc.Bacc("TRN2", target_bir_lowering=False, debug=False)
    r_ext = nc.declare_dram_parameter("r", [N, D], F32, isOutput=False)
    w1_ext = nc.declare_dram_parameter("W1", [D, H], F32, isOutput=False)
    b1_ext = nc.declare_dram_parameter("b1", [H], F32, isOutput=False)
    w2_ext = nc.declare_dram_parameter("W2", [H, O], F32, isOutput=False)
    b2_ext = nc.declare_dram_parameter("b2", [O], F32, isOutput=False)
    # transposed bf16 output [O, N]; host transposes + upcasts
    out_ext = nc.declare_dram_parameter("out", [O, N], BF16, isOutput=True)

    # DRAM bounce buffers (partition->free transposition staging, 16KB each)
    scr_nhsq_bf = nc.dram_tensor("scr_nhsq_bf", [NB, P], BF16)
    scr_sinv = nc.dram_tensor("scr_sinv", [NB, P], F32)

    def flat_row_ap(dram_t):
        a = dram_t[:, :].rearrange("a b -> (a b)")
        return bass.AP(tensor=a.tensor, offset=a.offset, ap=[[1, 1]] + list(a.ap))

    def col_ap(dram_1d, parts, nfree):
        # read 1-D dram tensor [parts*nfree] as [parts, nfree] column layout:
        # out[p, f] = t[f*parts + p]
        a = dram_1d[:]
        return bass.AP(
            tensor=a.tensor, offset=a.offset, ap=[[1, parts], [parts, nfree]]
        )

    with tile.TileContext(nc) as tc, ExitStack() as ctx:
        consts = ctx.enter_context(tc.tile_pool(name="consts", bufs=1))
        stage = ctx.enter_context(tc.tile_pool(name="stage", bufs=2))
        upool = ctx.enter_context(tc.tile_pool(name="upool", bufs=3))
        psA = ctx.enter_context(tc.tile_pool(name="psA", bufs=3, space="PSUM"))
        psY = ctx.enter_context(tc.tile_pool(name="psY", bufs=1, space="PSUM"))

        ident = consts.tile([P, P], F32)
        make_identity(nc, ident)

        ones_bf = consts.tile([1, P], BF16)
        nc.gpsimd.memset(ones_bf, 1.0)
        ones_f = consts.tile([1, P], F32)
        nc.gpsimd.memset(ones_f, 1.0)

        # ---- load & cast inputs ------------------------------------------
        # 4-block DMA groups across 3 queues; transposes on PE; psum copies
        # alternate DVE/ACT.
        GB = 4
        r_bf = consts.tile([P, NB, D], BF16)
        rT_bf = consts.tile([P, N], BF16)
        sqh_col = consts.tile([P, NB], F32)   # +sq/2
        r_src = r_ext[:, :].rearrange("(nb p) d -> p nb d", p=P)
        dqs = [nc.sync, nc.gpsimd, nc.scalar]
        for g0 in range(0, NB, GB):
            rld = upool.tile([P, GB, D], F32, tag="rld")
            dqs[(g0 // GB) % 3].dma_start(
                out=rld, in_=r_src[:, g0 : g0 + GB, :]
            )
            nc.vector.tensor_copy(out=r_bf[:, g0 : g0 + GB, :], in_=rld)
            for bi in range(GB):
                b = g0 + bi
                tp = psA.tile([P, QW], F32, tag="ps")
                nc.tensor.transpose(tp[:, :P], rld[:, bi, :], ident)
                if b % 2 == 0:
                    nc.vector.tensor_copy(
                        out=rT_bf[:, b * P : (b + 1) * P], in_=tp[:, :P]
                    )
                else:
                    nc.scalar.copy(
                        out=rT_bf[:, b * P : (b + 1) * P], in_=tp[:, :P]
                    )
                rsq = upool.tile([P, D], BF16, tag="rsq")
                # Square(r * sqrt(0.5)) = 0.5*r^2; accum -> sq/2 per partition
                nc.scalar.activation(
                    out=rsq,
                    in_=r_bf[:, b, :],
                    func=Act.Square,
                    bias=0.0,
                    scale=0.70710678,
                    accum_out=sqh_col[:, b : b + 1],
                )

        # FFN weights (replicated, small): loads on gpsimd queue
        w1f = stage.tile([P, H], F32, tag="wld")
        nc.gpsimd.dma_start(out=w1f, in_=w1_ext[:, :])
        w1_bf = consts.tile([P, H], BF16)
        nc.vector.tensor_copy(out=w1_bf, in_=w1f)

        w2f = stage.tile([P, HB, O], F32, tag="wld2")
        nc.gpsimd.dma_start(
            out=w2f, in_=w2_ext[:, :].rearrange("(hb p) o -> p hb o", p=P)
        )
        w2_bf = consts.tile([P, HB, O], BF16)
        nc.vector.tensor_copy(out=w2_bf, in_=w2f)

        b1_col = consts.tile([P, HB], F32)    # b1[hb*128+p]
        nc.gpsimd.dma_start(out=b1_col, in_=col_ap(b1_ext, P, HB))
        b2_col = consts.tile([P, 1], F32)
        nc.gpsimd.dma_start(out=b2_col, in_=col_ap(b2_ext, P, 1))

        # ---- sq machinery ------------------------------------------------
        nsq_col = consts.tile([P, NB], F32)   # -sq (Exp bias)
        nc.vector.tensor_scalar_mul(nsq_col, sqh_col, -2.0)
        nhsq_col = consts.tile([P, NB], F32)  # -sq/2
        nc.vector.tensor_scalar_mul(nhsq_col, sqh_col, -1.0)

        # transpose -sq/2 -> [NB, P], bounce 16KB through DRAM -> row [1, N]
        tpq = psA.tile([P, QW], F32, tag="ps")
        nc.tensor.transpose(tpq[:NB, :P], nhsq_col, ident)
        nhsqT_bf = stage.tile([NB, P], BF16, tag="nhsqT")
        nc.vector.tensor_copy(out=nhsqT_bf, in_=tpq[:NB, :P])
        nc.sync.dma_start(out=scr_nhsq_bf[:, :], in_=nhsqT_bf)
        nrow = consts.tile([1, N], BF16)
        nc.sync.dma_start(out=nrow, in_=flat_row_ap(scr_nhsq_bf))

        # ---- main loop: gram -> exp -> aggregate -------------------------
        s_slots = consts.tile([P, NB * NPASS], F32)
        ysb = consts.tile([P, N], F32)

        for qp in range(NPASS):
            base = qp * QW
            # bcq[p, j] = -sq_j/2 broadcast over partitions (rank-1 matmul)
            bc_ps = psA.tile([P, QW], F32, tag="ps")
            for c in range(CPQ):
                cs = slice(c * CH, (c + 1) * CH)
                nc.tensor.matmul(
                    bc_ps[:, cs],
                    lhsT=ones_bf,
                    rhs=nrow[0:1, base + c * CH : base + (c + 1) * CH],
                    start=True,
                    stop=True,
                )
            bcq = stage.tile([P, QW], F32, tag="bcq")
            nc.scalar.copy(out=bcq, in_=bc_ps)

            yt = psY.tile([P, QW], F32, tag="y")
            gtiles = [None] * NB
            utiles = [None] * NB

            def issue_gram(n):
                g = psA.tile([P, QW], F32, tag="ps")
                gtiles[n] = g
                ncol = slice(n * P, (n + 1) * P)
                for c in range(CPQ):
                    cs = slice(c * CH, (c + 1) * CH)
                    nc.tensor.matmul(
                        g[:, cs],
                        lhsT=rT_bf[:, ncol],
                        rhs=rT_bf[:, base + c * CH : base + (c + 1) * CH],
                        start=True,
                        stop=True,
                    )
                d2h = upool.tile([P, QW], BF16, tag="d2")
                nc.vector.tensor_tensor(out=d2h, in0=g, in1=bcq, op=Alu.add)
                u = upool.tile([P, QW], BF16, tag="u")
                utiles[n] = u
                slot = n * NPASS + qp
                nc.scalar.activation(
                    out=u,
                    in_=d2h,
                    func=Act.Exp,
                    bias=nsq_col[:, n : n + 1],
                    scale=2.0,
                    accum_out=s_slots[:, slot : slot + 1],
                )

            def issue_y(n):
                u = utiles[n]
                for c in range(CPQ):
                    cs = slice(c * CH, (c + 1) * CH)
                    nc.tensor.matmul(
                        yt[:, cs],
                        lhsT=r_bf[:, n, :],
                        rhs=u[:, cs],
                        start=(n == 0),
                        stop=(n == NB - 1),
                    )

            for k in range(NB + LA):
                if k < NB:
                    issue_gram(k)
                if k >= LA:
                    issue_y(k - LA)
            nc.vector.tensor_copy(out=ysb[:, base : base + QW], in_=yt)

        # ---- normalize + residual ----------------------------------------
        s_col = consts.tile([P, NB], F32)
        nc.vector.tensor_reduce(
            out=s_col,
            in_=s_slots.rearrange("p (nb t) -> p nb t", t=NPASS),
            axis=mybir.AxisListType.X,
            op=Alu.add,
        )
        sinv_col = consts.tile([P, NB], F32)
        nc.vector.reciprocal(out=sinv_col, in_=s_col)
        tps = psA.tile([P, QW], F32, tag="ps")
        nc.tensor.transpose(tps[:NB, :P], sinv_col, ident)
        sinvT_f = stage.tile([NB, P], F32, tag="sinvT")
        nc.vector.tensor_copy(out=sinvT_f, in_=tps[:NB, :P])
        nc.sync.dma_start(out=scr_sinv[:, :], in_=sinvT_f)
        srow = consts.tile([1, N], F32)
        nc.sync.dma_start(out=srow, in_=flat_row_ap(scr_sinv))

        r2 = consts.tile([P, N], BF16)
        hT = [
            consts.tile([P, N], BF16, name=f"hT{hb}", tag=f"hT{hb}")
            for hb in range(HB)
        ]

        for qp in range(NPASS):
            base = qp * QW
            # sinv broadcast chunk via rank-1 matmul (f32)
            sb_ps = psA.tile([P, QW], F32, tag="ps")
            for c in range(CPQ):
                cs = slice(c * CH, (c + 1) * CH)
                nc.tensor.matmul(
                    sb_ps[:, cs],
                    lhsT=ones_f,
                    rhs=srow[0:1, base + c * CH : base + (c + 1) * CH],
                    start=True,
                    stop=True,
                )
            # r2 per 512-chunk so fc1 can start on chunk 0 early
            r2t = stage.tile([P, QW], BF16, tag="r2t")
            for c in range(CPQ):
                cs = slice(base + c * CH, base + (c + 1) * CH)
                lcs = slice(c * CH, (c + 1) * CH)
                nc.vector.tensor_tensor(
                    out=r2t[:, lcs], in0=ysb[:, cs], in1=sb_ps[:, lcs],
                    op=Alu.mult,
                )
                nc.vector.tensor_tensor(
                    out=r2[:, cs], in0=r2t[:, lcs], in1=rT_bf[:, cs],
                    op=Alu.add,
                )

            # fc1: per-chunk MMs gated only on their r2 chunk
            for hb in range(HB):
                hp = psA.tile([P, QW], F32, tag="ps")
                for c in range(CPQ):
                    cs = slice(base + c * CH, base + (c + 1) * CH)
                    nc.tensor.matmul(
                        hp[:, c * CH : (c + 1) * CH],
                        lhsT=w1_bf[:, hb * P : (hb + 1) * P],
                        rhs=r2[:, cs],
                        start=True,
                        stop=True,
                    )
                nc.scalar.activation(
                    out=hT[hb][:, base : base + QW],
                    in_=hp,
                    func=Act.Lrelu,
                    bias=b1_col[:, hb : hb + 1],
                    scale=1.0,
                    alpha=0.01,
                )

            # fc2 for this chunk: outT[o, n] = sum_hb W2_hb.T @ hT_hb
            for c in range(CPQ):
                ncols = slice(base + c * CH, base + (c + 1) * CH)
                op = psA.tile([P, CH], F32, tag="ps")
                for hb in range(HB):
                    nc.tensor.matmul(
                        op,
                        lhsT=w2_bf[:, hb, :],
                        rhs=hT[hb][:, ncols],
                        start=(hb == 0),
                        stop=(hb == HB - 1),
                    )
                osb = upool.tile([P, CH], BF16, tag="osb")
                nc.vector.tensor_scalar_add(osb, op, b2_col[:, 0:1])
                (nc.sync if c % 2 == 0 else nc.gpsimd).dma_start(
                    out=out_ext[:, ncols], in_=osb
                )

    nc.compile()
    return nc


_NC_CACHE = {}


def _get_nc(**kw):
    key = tuple(sorted(kw.items()))
    if key not in _NC_CACHE:
        _NC_CACHE[key] = build_nc(**kw)
    return _NC_CACHE[key]


def kernel(r, W1, b1, W2, b2):
    r = np.ascontiguousarray(r, dtype=np.float32)
    W1 = np.ascontiguousarray(W1, dtype=np.float32)
    b1 = np.ascontiguousarray(b1, dtype=np.float32)
    W2 = np.ascontiguousarray(W2, dtype=np.float32)
    b2 = np.ascontiguousarray(b2, dtype=np.float32)
    B, N, D = r.shape
    assert (B, N, D) == (B_FULL, N_FULL, D_FULL)

    nc = _get_nc()
    in_maps = [
        {"r": r[i], "W1": W1, "b1": b1, "W2": W2, "b2": b2} for i in range(B)
    ]
    res = run_bass_kernel_spmd(nc, in_maps, list(range(N_CORES)))
    # out is bf16 [O, N] per core; transpose back + upcast
    return np.stack(
        [np.ascontiguousarray(res.results[i]["out"].astype(np.float32).T)
         for i in range(B)]
    )


if __name__ == "__main__":
    rng = np.random.default_rng(0)
    r = rng.standard_normal((B_FULL, N_FULL, D_FULL), dtype=np.float32)
    W1 = rng.standard_normal((D_FULL, H_FULL), dtype=np.float32) * 0.08
    b1 = rng.standard_normal((H_FULL,), dtype=np.float32) * 0.08
    W2 = rng.standard_normal((H_FULL, O_FULL), dtype=np.float32) * 0.04
    b2 = rng.standard_normal((O_FULL,), dtype=np.float32) * 0.04
    out = kernel(r=r, W1=W1, b1=b1, W2=W2, b2=b2)
    print(out.shape, out.dtype)


# revision 22
# speedup vs baseline: 1.7034x; 1.0309x over previous
"""Trainium2 Bass kernel for gnn_message_passing (nn_CMMLunit_50173807952434).

reference math (per batch sample, N=4096, D=128, H=512, O=128):
    d2[i,j] = ||r_i||^2 + ||r_j||^2 - 2 r_i.r_j   (clamped at 0)
    w = exp(-d2); w = w / rowsum(w); w = w + I
    r2 = w @ r
    out = leaky_relu(r2 @ W1 + b1, 0.01) @ W2 + b2

Sharding: data-parallel over batch B=8 across 8 cores (1 sample/core),
FFN weights replicated, no collectives.

v2 per-core pipeline (all matmuls bf16 into fp32 PSUM):
  - load r -> r_bf [128,(nb,128)]; rT_bf [128,N] via PE transposes
  - sq/2 via ACT Square accum; -sq_j/2 row [1,N] via 16KB DRAM bounce
  - per column pass qp (4 x 1024 cols):
      bcq[128,1024] = broadcast(-sq_j/2) via rank-1 PE matmul (once/pass)
      per row block n (32): PSUM g = rT_n.T @ rT_cols (2x512, one shared
        stationary); DVE tt: d2h = g + bcq (bf16); ACT: u =
        Exp(2*d2h - sq_i) with per-partition bias, accum -> row-sum slot;
        yT += r_n.T @ u (software-pipelined 2 iters behind gram)
  - s = sum slots; 1/s -> row [1,N] via 16KB bounce; rank-1 PE broadcast
    per chunk; r2T = yT*sinv + rT (DVE)
  - FFN: hT = Lrelu(W1.T@r2T + b1) via ACT bias+alpha;
    outT[o,n] = sum_hb W2_hb.T @ hT_hb (W2 stationary), +b2 via DVE,
    DMA outT [O,N] -> DRAM; host transposes.
"""

import numpy as np
from contextlib import ExitStack

import concourse.bass as bass
import concourse.bacc as bacc
import concourse.tile as tile
from concourse import mybir
from concourse.bass_utils import run_bass_kernel_spmd
from concourse.masks import make_identity

F32 = mybir.dt.float32
BF16 = mybir.dt.bfloat16
Alu = mybir.AluOpType
Act = mybir.ActivationFunctionType

P = 128  # partitions

# main problem dims (hardcoded; harness contract)
B_FULL, N_FULL, D_FULL = 8, 4096, 128
H_FULL, O_FULL = 512, 128
N_CORES = 8

USE_ACT_LRELU = True  # Lrelu on ACT (bias=b1, alpha=0.01); else copy+DVE stt


def build_nc(N=N_FULL, D=D_FULL, H=H_FULL, O=O_FULL):
    """Build the single-core Bass program (SPMD across cores)."""
    assert D == P
    NB = N // P              # row blocks
    HB = H // P
    QW = min(1024, N)        # gram/ACT tile width (2 psum banks)
    NPASS = N // QW          # column passes
    CH = 512                 # matmul chunk (one psum bank)
    CPQ = QW // CH
    LA = 2                   # y-matmul lookahead (software pipeline depth)

    nc = bacc.Bacc("TRN2", target_bir_lowering=False, debug=False)
    r_ext = nc.declare_dram_parameter("r", [N, D], F32, isOutput=False)
    w1_ext = nc.declare_dram_parameter("W1", [D, H], F32, isOutput=False)
    b1_ext = nc.declare_dram_parameter("b1", [H], F32, isOutput=False)
    w2_ext = nc.declare_dram_parameter("W2", [H, O], F32, isOutput=False)
    b2_ext = nc.declare_dram_parameter("b2", [O], F32, isOutput=False)
    # transposed bf16 output [O, N]; host transposes + upcasts
    out_ext = nc.declare_dram_parameter("out", [O, N], BF16, isOutput=True)

    # DRAM bounce buffers (partition->free transposition staging, 16KB each)
    scr_nhsq_bf = nc.dram_tensor("scr_nhsq_bf", [NB, P], BF16)
    scr_sinv = nc.dram_tensor("scr_sinv", [NB, P], F32)

    def flat_row_ap(dram_t):
        a = dram_t[:, :].rearrange("a b -> (a b)")
        return bass.AP(tensor=a.tensor, offset=a.offset, ap=[[1, 1]] + list(a.ap))

    def col_ap(dram_1d, parts, nfree):
        # read 1-D dram tensor [parts*nfree] as [parts, nfree] column layout:
        # out[p, f] = t[f*parts + p]
        a = dram_1d[:]
        return bass.AP(
            tensor=a.tensor, offset=a.offset, ap=[[1, parts], [parts, nfree]]
        )

    with tile.TileContext(nc) as tc, ExitStack() as ctx:
        consts = ctx.enter_context(tc.tile_pool(name="consts", bufs=1))
        stage = ctx.enter_context(tc.tile_pool(name="stage", bufs=2))
        upool = ctx.enter_context(tc.tile_pool(name="upool", bufs=3))
        psA = ctx.enter_context(tc.tile_pool(name="psA", bufs=3, space="PSUM"))
        psY = ctx.enter_context(tc.tile_pool(name="psY", bufs=1, space="PSUM"))

        ident = consts.tile([P, P], F32)
        make_identity(nc, ident)

        ones_bf = consts.tile([1, P], BF16)
        nc.gpsimd.memset(ones_bf, 1.0)
        ones_f = consts.tile([1, P], F32)
        nc.gpsimd.memset(ones_f, 1.0)

        # ---- load & cast inputs ------------------------------------------
        # 4-block DMA groups across 3 queues; transposes on PE; psum copies
        # alternate DVE/ACT.
        GB = 4
        r_bf = consts.tile([P, NB, D], BF16)
        rT_bf = consts.tile([P, N], BF16)
        rsq_all = consts.tile([P, NB, D], BF16)  # r_bf^2 (for sq reduce)
        r_src = r_ext[:, :].rearrange("(nb p) d -> p nb d", p=P)
        dqs = [nc.sync, nc.gpsimd, nc.scalar]
        for g0 in range(0, NB, GB):
            rld = upool.tile([P, GB, D], F32, tag="rld")
            dqs[(g0 // GB) % 3].dma_start(
                out=rld, in_=r_src[:, g0 : g0 + GB, :]
            )
            nc.vector.tensor_copy(out=r_bf[:, g0 : g0 + GB, :], in_=rld)
            nc.vector.tensor_tensor(
                out=rsq_all[:, g0 : g0 + GB, :],
                in0=r_bf[:, g0 : g0 + GB, :],
                in1=r_bf[:, g0 : g0 + GB, :],
                op=Alu.mult,
            )
            for bi in range(GB):
                b = g0 + bi
                tp = psA.tile([P, QW], F32, tag="ps")
                nc.tensor.transpose(tp[:, :P], rld[:, bi, :], ident)
                # rT copies on ACT (DVE carries the casts/squares)
                nc.scalar.copy(
                    out=rT_bf[:, b * P : (b + 1) * P], in_=tp[:, :P]
                )
        # sq per (partition, block) in one drain-free reduce
        sq_col = consts.tile([P, NB], F32)
        nc.vector.tensor_reduce(
            out=sq_col,
            in_=rsq_all,
            axis=mybir.AxisListType.X,
            op=Alu.add,
        )

        # FFN weights (replicated, small): loads on gpsimd queue
        w1f = stage.tile([P, H], F32, tag="wld")
        nc.gpsimd.dma_start(out=w1f, in_=w1_ext[:, :])
        w1_bf = consts.tile([P, H], BF16)
        nc.vector.tensor_copy(out=w1_bf, in_=w1f)

        w2f = stage.tile([P, HB, O], F32, tag="wld2")
        nc.gpsimd.dma_start(
            out=w2f, in_=w2_ext[:, :].rearrange("(hb p) o -> p hb o", p=P)
        )
        w2_bf = consts.tile([P, HB, O], BF16)
        nc.vector.tensor_copy(out=w2_bf, in_=w2f)

        b1_col = consts.tile([P, HB], F32)    # b1[hb*128+p]
        nc.gpsimd.dma_start(out=b1_col, in_=col_ap(b1_ext, P, HB))
        b2_col = consts.tile([P, 1], F32)
        nc.gpsimd.dma_start(out=b2_col, in_=col_ap(b2_ext, P, 1))

        # ---- sq machinery ------------------------------------------------
        nsq_col = consts.tile([P, NB], F32)   # -sq (Exp bias)
        nc.vector.tensor_scalar_mul(nsq_col, sq_col, -1.0)
        nhsq_col = consts.tile([P, NB], F32)  # -sq/2
        nc.vector.tensor_scalar_mul(nhsq_col, sq_col, -0.5)

        # transpose -sq/2 -> [NB, P], bounce 16KB through DRAM -> row [1, N]
        tpq = psA.tile([P, QW], F32, tag="ps")
        nc.tensor.transpose(tpq[:NB, :P], nhsq_col, ident)
        nhsqT_bf = stage.tile([NB, P], BF16, tag="nhsqT")
        nc.vector.tensor_copy(out=nhsqT_bf, in_=tpq[:NB, :P])
        nc.sync.dma_start(out=scr_nhsq_bf[:, :], in_=nhsqT_bf)
        nrow = consts.tile([1, N], BF16)
        nc.sync.dma_start(out=nrow, in_=flat_row_ap(scr_nhsq_bf))

        # ---- main loop: gram -> exp -> aggregate -------------------------
        s_slots = consts.tile([P, NB * NPASS], F32)
        ysb = consts.tile([P, N], F32)

        for qp in range(NPASS):
            base = qp * QW
            # bcq[p, j] = -sq_j/2 broadcast over partitions (rank-1 matmul)
            bc_ps = psA.tile([P, QW], F32, tag="ps")
            for c in range(CPQ):
                cs = slice(c * CH, (c + 1) * CH)
                nc.tensor.matmul(
                    bc_ps[:, cs],
                    lhsT=ones_bf,
                    rhs=nrow[0:1, base + c * CH : base + (c + 1) * CH],
                    start=True,
                    stop=True,
                )
            bcq = stage.tile([P, QW], F32, tag="bcq")
            nc.scalar.copy(out=bcq, in_=bc_ps)

            yt = psY.tile([P, QW], F32, tag="y")
            gtiles = [None] * NB
            utiles = [None] * NB

            def issue_gram(n):
                g = psA.tile([P, QW], F32, tag="ps")
                gtiles[n] = g
                ncol = slice(n * P, (n + 1) * P)
                for c in range(CPQ):
                    cs = slice(c * CH, (c + 1) * CH)
                    nc.tensor.matmul(
                        g[:, cs],
                        lhsT=rT_bf[:, ncol],
                        rhs=rT_bf[:, base + c * CH : base + (c + 1) * CH],
                        start=True,
                        stop=True,
                    )
                d2h = upool.tile([P, QW], BF16, tag="d2")
                nc.vector.tensor_tensor(out=d2h, in0=g, in1=bcq, op=Alu.add)
                u = upool.tile([P, QW], BF16, tag="u")
                utiles[n] = u
                slot = n * NPASS + qp
                nc.scalar.activation(
                    out=u,
                    in_=d2h,
                    func=Act.Exp,
                    bias=nsq_col[:, n : n + 1],
                    scale=2.0,
                    accum_out=s_slots[:, slot : slot + 1],
                )

            def issue_y(n):
                u = utiles[n]
                for c in range(CPQ):
                    cs = slice(c * CH, (c + 1) * CH)
                    nc.tensor.matmul(
                        yt[:, cs],
                        lhsT=r_bf[:, n, :],
                        rhs=u[:, cs],
                        start=(n == 0),
                        stop=(n == NB - 1),
                    )

            for k in range(NB + LA):
                if k < NB:
                    issue_gram(k)
                if k >= LA:
                    issue_y(k - LA)
            nc.vector.tensor_copy(out=ysb[:, base : base + QW], in_=yt)

        # ---- normalize + residual ----------------------------------------
        s_col = consts.tile([P, NB], F32)
        nc.vector.tensor_reduce(
            out=s_col,
            in_=s_slots.rearrange("p (nb t) -> p nb t", t=NPASS),
            axis=mybir.AxisListType.X,
            op=Alu.add,
        )
        sinv_col = consts.tile([P, NB], F32)
        nc.vector.reciprocal(out=sinv_col, in_=s_col)
        tps = psA.tile([P, QW], F32, tag="ps")
        nc.tensor.transpose(tps[:NB, :P], sinv_col, ident)
        sinvT_f = stage.tile([NB, P], F32, tag="sinvT")
        nc.vector.tensor_copy(out=sinvT_f, in_=tps[:NB, :P])
        nc.sync.dma_start(out=scr_sinv[:, :], in_=sinvT_f)
        srow = consts.tile([1, N], F32)
        nc.sync.dma_start(out=srow, in_=flat_row_ap(scr_sinv))

        # warm-keeper: the sinv bounce leaves the PE idle ~3.4us, exactly one
        # HAM MID window -> the whole FFN would run at 1.2 GHz. A short
        # throwaway accumulation (with one tiny consumer) bridges the gap.
        NDUMMY = 10
        dmy = psY.tile([P, CH], F32, tag="y")
        for i in range(NDUMMY):
            nc.tensor.matmul(
                dmy,
                lhsT=rT_bf[:, 0:P],
                rhs=rT_bf[:, 0:CH],
                start=(i == 0),
                stop=(i == NDUMMY - 1),
            )
        dsb = stage.tile([1, 8], F32, tag="dsb")
        nc.vector.tensor_copy(out=dsb, in_=dmy[0:1, 0:8])
        nc.sync.dma_start(out=scr_sinv[0:1, 0:8], in_=dsb)

        r2 = consts.tile([P, N], BF16)
        hT = [
            consts.tile([P, N], BF16, name=f"hT{hb}", tag=f"hT{hb}")
            for hb in range(HB)
        ]

        for qp in range(NPASS):
            base = qp * QW
            # sinv broadcast chunk via rank-1 matmul (f32)
            sb_ps = psA.tile([P, QW], F32, tag="ps")
            for c in range(CPQ):
                cs = slice(c * CH, (c + 1) * CH)
                nc.tensor.matmul(
                    sb_ps[:, cs],
                    lhsT=ones_f,
                    rhs=srow[0:1, base + c * CH : base + (c + 1) * CH],
                    start=True,
                    stop=True,
                )
            # r2 per 512-chunk so fc1 can start on chunk 0 early
            r2t = stage.tile([P, QW], BF16, tag="r2t")
            for c in range(CPQ):
                cs = slice(base + c * CH, base + (c + 1) * CH)
                lcs = slice(c * CH, (c + 1) * CH)
                nc.vector.tensor_tensor(
                    out=r2t[:, lcs], in0=ysb[:, cs], in1=sb_ps[:, lcs],
                    op=Alu.mult,
                )
                nc.vector.tensor_tensor(
                    out=r2[:, cs], in0=r2t[:, lcs], in1=rT_bf[:, cs],
                    op=Alu.add,
                )

            # fc1: per-chunk MMs gated only on their r2 chunk
            for hb in range(HB):
                hp = psA.tile([P, QW], F32, tag="ps")
                for c in range(CPQ):
                    cs = slice(base + c * CH, base + (c + 1) * CH)
                    nc.tensor.matmul(
                        hp[:, c * CH : (c + 1) * CH],
                        lhsT=w1_bf[:, hb * P : (hb + 1) * P],
                        rhs=r2[:, cs],
                        start=True,
                        stop=True,
                    )
                nc.scalar.activation(
                    out=hT[hb][:, base : base + QW],
                    in_=hp,
                    func=Act.Lrelu,
                    bias=b1_col[:, hb : hb + 1],
                    scale=1.0,
                    alpha=0.01,
                )

            # fc2 for this chunk: outT[o, n] = sum_hb W2_hb.T @ hT_hb
            for c in range(CPQ):
                ncols = slice(base + c * CH, base + (c + 1) * CH)
                op = psA.tile([P, CH], F32, tag="ps")
                for hb in range(HB):
                    nc.tensor.matmul(
                        op,
                        lhsT=w2_bf[:, hb, :],
                        rhs=hT[hb][:, ncols],
                        start=(hb == 0),
                        stop=(hb == HB - 1),
                    )
                osb = upool.tile([P, CH], BF16, tag="osb")
                nc.vector.tensor_scalar_add(osb, op, b2_col[:, 0:1])
                (nc.sync if c % 2 == 0 else nc.gpsimd).dma_start(
                    out=out_ext[:, ncols], in_=osb
                )

    nc.compile()
    return nc


_NC_CACHE = {}


def _get_nc(**kw):
    key = tuple(sorted(kw.items()))
    if key not in _NC_CACHE:
        _NC_CACHE[key] = build_nc(**kw)
    return _NC_CACHE[key]


def kernel(r, W1, b1, W2, b2):
    r = np.ascontiguousarray(r, dtype=np.float32)
    W1 = np.ascontiguousarray(W1, dtype=np.float32)
    b1 = np.ascontiguousarray(b1, dtype=np.float32)
    W2 = np.ascontiguousarray(W2, dtype=np.float32)
    b2 = np.ascontiguousarray(b2, dtype=np.float32)
    B, N, D = r.shape
    assert (B, N, D) == (B_FULL, N_FULL, D_FULL)

    nc = _get_nc()
    in_maps = [
        {"r": r[i], "W1": W1, "b1": b1, "W2": W2, "b2": b2} for i in range(B)
    ]
    res = run_bass_kernel_spmd(nc, in_maps, list(range(N_CORES)))
    # out is bf16 [O, N] per core; transpose back + upcast
    return np.stack(
        [np.ascontiguousarray(res.results[i]["out"].astype(np.float32).T)
         for i in range(B)]
    )


if __name__ == "__main__":
    rng = np.random.default_rng(0)
    r = rng.standard_normal((B_FULL, N_FULL, D_FULL), dtype=np.float32)
    W1 = rng.standard_normal((D_FULL, H_FULL), dtype=np.float32) * 0.08
    b1 = rng.standard_normal((H_FULL,), dtype=np.float32) * 0.08
    W2 = rng.standard_normal((H_FULL, O_FULL), dtype=np.float32) * 0.04
    b2 = rng.standard_normal((O_FULL,), dtype=np.float32) * 0.04
    out = kernel(r=r, W1=W1, b1=b1, W2=W2, b2=b2)
    print(out.shape, out.dtype)
